# revision 7
# baseline (speedup 1.0000x reference)
"""Causal multi-head attention (B=2, S=2048, D=1024, H=16) on 8 Trainium2
NeuronCores.

Sharding: core c = (batch b = c//4) x (head-group g = c%4, 4 heads each).
Per core:
  - QKV projections for its 4 heads, computed in transposed layout
    (QT/KT: [head-cols, S]) plus V in natural layout [S, head-cols].
  - Flash-style causal attention per head with scores computed transposed
    (ST[k, q]) so softmax normalization lands on PE/ACT/DVE cheaply:
    exp without max-subtraction (scores are O(1) by construction), row sums
    via an appended ones-column on V, division via reciprocal_approx_fast
    plus a PE broadcast.
  - AllGather (groups of 4 cores sharing a batch) of the normalized,
    transposed attention output, then a column-parallel output projection.
Host side only shards inputs / concatenates outputs.

All matmuls run as float32r (1-pass fp32, ~FP22 multiply, fp32 accumulate).
"""
import sys

if "/opt/trn_rl_repo" not in sys.path:
    sys.path.insert(0, "/opt/trn_rl_repo")

import numpy as np

import concourse.bass as bass
import concourse.mybir as mybir
import concourse.tile as tile
from concourse import bacc
from concourse.bass import ts
from concourse.bass_utils import run_bass_kernel_spmd

F32 = mybir.dt.float32
F32R = mybir.dt.float32r
AF = mybir.ActivationFunctionType

MODEL_DIM = 1024
NUM_HEADS = 16
HEAD_DIM = 64
B = 2
S = 2048
N_CORES = 8
N_GROUPS = 4            # head groups (tensor parallel within a batch)
HPG = NUM_HEADS // N_GROUPS   # heads per group = 4
GC = HPG * HEAD_DIM     # head-group feature width = 256
QB = 512                # query block (PSUM bank free size)
NQB = S // QB           # 4
KT = 128                # key tile
NKT = S // KT           # 16
ND = MODEL_DIM // 128   # 8 contraction tiles for the projections

REPLICA_GROUPS = [[0, 1, 2, 3], [4, 5, 6, 7]]




def build_program(repeat: int = 1, debug: bool = False):
    """Build the SPMD per-core program. `repeat` re-runs the whole body
    (identical work) for wall-clock loop-differenced timing."""
    nc = bacc.Bacc("TRN2", num_devices=N_CORES)

    xt = nc.dram_tensor("xt", [MODEL_DIM, S], F32, kind="ExternalInput")
    wq = nc.dram_tensor("wq", [MODEL_DIM, GC], F32, kind="ExternalInput")
    wk = nc.dram_tensor("wk", [MODEL_DIM, GC], F32, kind="ExternalInput")
    wv = nc.dram_tensor("wv", [MODEL_DIM, GC], F32, kind="ExternalInput")
    wo = nc.dram_tensor("wo", [MODEL_DIM, GC], F32, kind="ExternalInput")
    bq = nc.dram_tensor("bq", [GC, 1], F32, kind="ExternalInput")   # already /8
    bk = nc.dram_tensor("bk", [GC, 1], F32, kind="ExternalInput")
    bo = nc.dram_tensor("bo", [1, GC], F32, kind="ExternalInput")   # bo + bv@Wo
    out = nc.dram_tensor("out", [S, GC], F32, kind="ExternalOutput")

    # internal DRAM for the collective, one pair per query block
    loc = [nc.dram_tensor(f"loc{i}", [GC, QB], F32) for i in range(NQB)]
    ag = [nc.dram_tensor(f"ag{i}", [MODEL_DIM, QB], F32) for i in range(NQB)]

    if debug:
        dbg_qt = nc.dram_tensor("dbg_qt", [128, 2 * S], F32, kind="ExternalOutput")
        dbg_kt = nc.dram_tensor("dbg_kt", [128, 2 * S], F32, kind="ExternalOutput")
        dbg_v = nc.dram_tensor("dbg_v", [128, NKT * HPG * (HEAD_DIM + 1)], F32,
                               kind="ExternalOutput")
        dbg_po = nc.dram_tensor("dbg_po", [65, QB], F32, kind="ExternalOutput")
        dbg_at = nc.dram_tensor("dbg_at", [65, QB], F32, kind="ExternalOutput")
        dbg_pt = nc.dram_tensor("dbg_pt", [128, QB], F32, kind="ExternalOutput")

    with tile.TileContext(nc) as tc:
        with (
            tc.tile_pool(name="const", bufs=1) as cpool,
            tc.tile_pool(name="big", bufs=1) as big,
            tc.tile_pool(name="work", bufs=1) as work,
            tc.tile_pool(name="ps", bufs=1, space="PSUM") as ps,
        ):
            # ---- constants ----
            ones64 = cpool.tile([128, 64], F32)      # row 64 = 1.0 (bcast lhsT)
            nc.gpsimd.memset(ones64[:], 0.0)
            nc.gpsimd.memset(ones64[64:65, :], 1.0)
            ones_r = cpool.tile([1, 128], F32)       # row 0 = 1.0 (bo bcast)
            nc.gpsimd.memset(ones_r[:], 1.0)
            tri = cpool.tile([128, 128], F32)        # 1 where col >= row
            nc.gpsimd.memset(tri[:], 1.0)
            nc.gpsimd.affine_select(
                out=tri[:], in_=tri[:], compare_op=mybir.AluOpType.is_ge,
                fill=0.0, base=0, pattern=[[1, 128]], channel_multiplier=-1)

            bo_row = cpool.tile([1, GC], F32)
            nc.sync.dma_start(bo_row[:], bo[:])
            bo_ps = ps.tile([128, GC], F32, tag="b")
            nc.tensor.matmul(bo_ps[:], ones_r[:, :], bo_row[:],
                             start=True, stop=True)
            bo_bc = cpool.tile([128, GC], F32)
            nc.scalar.copy(bo_bc[:], bo_ps[:])

            bq_sb = cpool.tile([128, 2, 1], F32)
            bk_sb = cpool.tile([128, 2, 1], F32)
            for ct in range(2):
                nc.sync.dma_start(bq_sb[:, ct, :], bq[ts(ct, 128), :])
                nc.sync.dma_start(bk_sb[:, ct, :], bk[ts(ct, 128), :])

            # ---- resident weights/activations ----
            xt_sb = big.tile([128, ND, S], F32R)
            wq_sb = big.tile([128, ND, GC], F32R)
            wk_sb = big.tile([128, ND, GC], F32R)
            wv_sb = big.tile([128, ND, GC], F32R)
            wo_sb = big.tile([128, ND, GC], F32R)
            for d in range(ND):
                nc.sync.dma_start(xt_sb[:, d, :], xt[ts(d, 128), :].bitcast(F32R))
                nc.sync.dma_start(wq_sb[:, d, :], wq[ts(d, 128), :].bitcast(F32R))
                nc.sync.dma_start(wk_sb[:, d, :], wk[ts(d, 128), :].bitcast(F32R))
                nc.sync.dma_start(wv_sb[:, d, :], wv[ts(d, 128), :].bitcast(F32R))
                nc.sync.dma_start(wo_sb[:, d, :], wo[ts(d, 128), :].bitcast(F32R))

            qt_sb = big.tile([128, 2, S], F32R)   # QT/8 (+bq/8): [col, s]
            kt_sb = big.tile([128, 2, S], F32R)
            v_sb = big.tile([128, NKT, HPG, HEAD_DIM + 1], F32R)  # V | ones
            ones_v = cpool.tile([128, NKT, HPG, 1], F32)
            nc.gpsimd.memset(ones_v[:], 1.0)
            nc.vector.tensor_copy(v_sb[:, :, :, HEAD_DIM:HEAD_DIM + 1], ones_v[:])

            def body():
                # ---- QKV projections ----
                for ct in range(2):
                    for sb in range(NQB):
                        pq = ps.tile([128, QB], F32, tag="s")
                        for d in range(ND):
                            nc.tensor.matmul(
                                pq[:], wq_sb[:, d, ts(ct, 128)],
                                xt_sb[:, d, ts(sb, QB)],
                                start=(d == 0), stop=(d == ND - 1))
                        nc.scalar.activation(
                            qt_sb[:, ct, ts(sb, QB)], pq[:], AF.Identity,
                            bias=bq_sb[:, ct, :], scale=0.125)
                        pk = ps.tile([128, QB], F32, tag="s")
                        for d in range(ND):
                            nc.tensor.matmul(
                                pk[:], wk_sb[:, d, ts(ct, 128)],
                                xt_sb[:, d, ts(sb, QB)],
                                start=(d == 0), stop=(d == ND - 1))
                        nc.scalar.activation(
                            kt_sb[:, ct, ts(sb, QB)], pk[:], AF.Identity,
                            bias=bk_sb[:, ct, :], scale=1.0)
                for st in range(NKT):
                    pv = ps.tile([128, GC], F32, tag="b")
                    for d in range(ND):
                        nc.tensor.matmul(
                            pv[:], xt_sb[:, d, ts(st, 128)],
                            wv_sb[:, d, :],
                            start=(d == 0), stop=(d == ND - 1))
                    nc.vector.tensor_copy(
                        v_sb[:, st, :, 0:HEAD_DIM],
                        pv[:].rearrange("p (h d) -> p h d", h=HPG))

                # ---- attention + collective + output projection ----
                for qb in range(NQB):
                    nkt_q = 4 * qb + 4       # causal: k tiles 0..4qb+3
                    for ct in range(2):
                        for hl in range(2):
                            h = ct * 2 + hl
                            pr = slice(64 * hl, 64 * hl + 64)
                            po = ps.tile([65, QB], F32, tag="o")
                            for kt in range(nkt_q):
                                dg = kt - 4 * qb
                                c0 = max(dg, 0) * 128
                                cols = slice(c0, QB)
                                pst = ps.tile([128, QB], F32, tag="s")
                                nc.tensor.matmul(
                                    pst[:, cols],
                                    kt_sb[pr, ct, ts(kt, KT)],
                                    qt_sb[pr, ct, qb * QB + c0:(qb + 1) * QB],
                                    start=True, stop=True,
                                    tile_position=(64 * hl, 0))
                                pt = work.tile([128, QB], F32R, tag="pt", bufs=4)
                                nc.scalar.activation(
                                    pt[:, cols], pst[:, cols], AF.Exp)
                                if dg >= 0:
                                    nc.vector.tensor_mul(
                                        pt[:, c0:c0 + 128],
                                        pt[:, c0:c0 + 128], tri[:])
                                if debug and qb == 0 and h == 0 and kt == 0:
                                    nc.sync.dma_start(dbg_pt[:], pt[:].bitcast(F32))
                                nc.tensor.matmul(
                                    po[:, cols], v_sb[:, kt, h, :],
                                    pt[:, cols],
                                    start=(kt == 0), stop=(kt == nkt_q - 1),
                                    skip_group_check=True)
                            at = work.tile([65, QB], F32, tag="at", bufs=3)
                            if debug and qb == 0 and h == 0:
                                po_sb = work.tile([65, QB], F32, tag="dbg", bufs=1)
                                nc.scalar.copy(po_sb[:], po[:])
                                nc.sync.dma_start(dbg_po[:], po_sb[:])
                            nc.scalar.copy(at[0:64, :], po[0:64, :])
                            rc = work.tile([65, QB], F32, tag="rc", bufs=2)
                            nc.vector.reciprocal_approx_fast(rc[:], po[:])
                            pb = ps.tile([64, QB], F32, tag="b")
                            nc.tensor.matmul(
                                pb[:], ones64[64:65, 0:64], rc[64:65, :],
                                start=True, stop=True)
                            nc.vector.tensor_mul(at[0:64, :], at[0:64, :], pb[:])
                            if debug and qb == 0 and h == 0:
                                nc.sync.dma_start(dbg_at[0:64, :], at[0:64, :])
                                nc.sync.dma_start(dbg_at[64:65, :], rc[64:65, :])
                            nc.sync.dma_start(
                                loc[qb][ts(h, 64), :], at[0:64, :])
                    nc.gpsimd.collective_compute(
                        "AllGather", mybir.AluOpType.bypass,
                        replica_groups=REPLICA_GROUPS,
                        ins=[loc[qb][:]], outs=[ag[qb][:]])
                    agt = work.tile([128, ND, QB], F32R, tag="ag", bufs=1)
                    for ft in range(ND):
                        nc.sync.dma_start(agt[:, ft, :], ag[qb][ts(ft, 128), :].bitcast(F32R))
                    for st4 in range(QB // 128):
                        pp = ps.tile([128, GC], F32, tag="o")
                        for ft in range(ND):
                            nc.tensor.matmul(
                                pp[:], agt[:, ft, ts(st4, 128)],
                                wo_sb[:, ft, :],
                                start=(ft == 0), stop=(ft == ND - 1))
                        ot = work.tile([128, GC], F32, tag="ot", bufs=3)
                        nc.vector.tensor_add(ot[:], pp[:], bo_bc[:])
                        nc.sync.dma_start(
                            out[qb * QB + st4 * 128:qb * QB + (st4 + 1) * 128, :],
                            ot[:])

            if repeat == 1:
                body()
                if debug:
                    nc.sync.dma_start(dbg_qt[:], qt_sb[:].rearrange("p a b -> p (a b)").bitcast(F32))
                    nc.sync.dma_start(dbg_kt[:], kt_sb[:].rearrange("p a b -> p (a b)").bitcast(F32))
                    nc.sync.dma_start(dbg_v[:], v_sb[:].rearrange("p a b c -> p (a b c)").bitcast(F32))
            else:
                with tc.For_i(0, repeat, 1):
                    body()

    nc.compile()
    return nc


def shard_inputs(X, Wq, bq, Wk, bk, Wv, bv, Wo, bo):
    """Full inputs -> per-core input maps."""
    in_maps = []
    for c in range(N_CORES):
        b, g = c // N_GROUPS, c % N_GROUPS
        cs = slice(g * GC, (g + 1) * GC)
        bo_eff = bo[cs] + bv.astype(np.float64) @ Wo[:, cs].astype(np.float64)
        in_maps.append({
            "xt": np.ascontiguousarray(X[b].T),
            "wq": np.ascontiguousarray(Wq[:, cs]),
            "wk": np.ascontiguousarray(Wk[:, cs]),
            "wv": np.ascontiguousarray(Wv[:, cs]),
            "wo": np.ascontiguousarray(Wo[:, cs]),
            "bq": np.ascontiguousarray((bq[cs] / 8.0).reshape(GC, 1)),
            "bk": np.ascontiguousarray(bk[cs].reshape(GC, 1)),
            "bo": np.ascontiguousarray(bo_eff.astype(np.float32).reshape(1, GC)),
        })
    return in_maps


def assemble_output(results):
    """Per-core output maps -> full [B, S, D]."""
    full = np.empty((B, S, MODEL_DIM), dtype=np.float32)
    for c in range(N_CORES):
        b, g = c // N_GROUPS, c % N_GROUPS
        full[b, :, g * GC:(g + 1) * GC] = results[c]["out"]
    return full


_prog_cache = {}


def kernel(X, Wq, bq, Wk, bk, Wv, bv, Wo, bo):
    X = np.asarray(X, dtype=np.float32)
    args = [np.asarray(a, dtype=np.float32)
            for a in (Wq, bq, Wk, bk, Wv, bv, Wo, bo)]
    if "nc" not in _prog_cache:
        _prog_cache["nc"] = build_program(repeat=1)
    nc = _prog_cache["nc"]
    in_maps = shard_inputs(X, *args)
    res = run_bass_kernel_spmd(nc, in_maps, list(range(N_CORES)))
    return assemble_output(res.results)


# revision 9
# speedup vs baseline: 1.2703x; 1.2703x over previous
"""Causal multi-head attention (B=2, S=2048, D=1024, H=16) on 8 Trainium2
NeuronCores.

Sharding: core c = (batch b = c//4) x (head-group g = c%4, 4 heads each).
Per core:
  - QKV projections for its 4 heads, computed in transposed layout
    (QT/KT: [head-cols, S]) plus V in natural layout [S, head-cols].
  - Flash-style causal attention per head with scores computed transposed
    (ST[k, q]) so softmax normalization lands on PE/ACT/DVE cheaply:
    exp without max-subtraction (scores are O(1) by construction), row sums
    via an appended ones-column on V, division via reciprocal_approx_fast
    plus a PE broadcast.
  - AllGather (groups of 4 cores sharing a batch) of the normalized,
    transposed attention output, then a column-parallel output projection.
Host side only shards inputs / concatenates outputs.

All matmuls run as float32r (1-pass fp32, ~FP22 multiply, fp32 accumulate).
"""
import sys

if "/opt/trn_rl_repo" not in sys.path:
    sys.path.insert(0, "/opt/trn_rl_repo")

import numpy as np

import concourse.bass as bass
import concourse.mybir as mybir
import concourse.tile as tile
from concourse import bacc
from concourse.bass import ts
from concourse.bass_utils import run_bass_kernel_spmd

F32 = mybir.dt.float32
F32R = mybir.dt.float32r
AF = mybir.ActivationFunctionType

MODEL_DIM = 1024
NUM_HEADS = 16
HEAD_DIM = 64
B = 2
S = 2048
N_CORES = 8
N_GROUPS = 4            # head groups (tensor parallel within a batch)
HPG = NUM_HEADS // N_GROUPS   # heads per group = 4
GC = HPG * HEAD_DIM     # head-group feature width = 256
QB = 512                # query block (PSUM bank free size)
NQB = S // QB           # 4
KT = 128                # key tile
NKT = S // KT           # 16
ND = MODEL_DIM // 128   # 8 contraction tiles for the projections

REPLICA_GROUPS = [[0, 1, 2, 3], [4, 5, 6, 7]]




def build_program(repeat: int = 1, debug: bool = False):
    """Build the SPMD per-core program. `repeat` re-runs the whole body
    (identical work) for wall-clock loop-differenced timing."""
    nc = bacc.Bacc("TRN2", num_devices=N_CORES)

    xt = nc.dram_tensor("xt", [MODEL_DIM, S], F32, kind="ExternalInput")
    wq = nc.dram_tensor("wq", [MODEL_DIM, GC], F32, kind="ExternalInput")
    wk = nc.dram_tensor("wk", [MODEL_DIM, GC], F32, kind="ExternalInput")
    wv = nc.dram_tensor("wv", [MODEL_DIM, GC], F32, kind="ExternalInput")
    wo = nc.dram_tensor("wo", [MODEL_DIM, GC], F32, kind="ExternalInput")
    bq = nc.dram_tensor("bq", [GC, 1], F32, kind="ExternalInput")   # already /8
    bk = nc.dram_tensor("bk", [GC, 1], F32, kind="ExternalInput")
    bo = nc.dram_tensor("bo", [1, GC], F32, kind="ExternalInput")   # bo + bv@Wo
    out = nc.dram_tensor("out", [S, GC], F32, kind="ExternalOutput")

    # internal DRAM for the collective, one pair per query block
    loc = [nc.dram_tensor(f"loc{i}", [GC, QB], F32) for i in range(NQB)]
    ag = [nc.dram_tensor(f"ag{i}", [MODEL_DIM, QB], F32) for i in range(NQB)]

    if debug:
        dbg_qt = nc.dram_tensor("dbg_qt", [128, 2 * S], F32, kind="ExternalOutput")
        dbg_kt = nc.dram_tensor("dbg_kt", [128, 2 * S], F32, kind="ExternalOutput")
        dbg_v = nc.dram_tensor("dbg_v", [128, NKT * HPG * (HEAD_DIM + 1)], F32,
                               kind="ExternalOutput")
        dbg_po = nc.dram_tensor("dbg_po", [65, QB], F32, kind="ExternalOutput")
        dbg_at = nc.dram_tensor("dbg_at", [65, QB], F32, kind="ExternalOutput")
        dbg_pt = nc.dram_tensor("dbg_pt", [128, QB], F32, kind="ExternalOutput")

    with tile.TileContext(nc) as tc:
        with (
            tc.tile_pool(name="const", bufs=1) as cpool,
            tc.tile_pool(name="big", bufs=1) as big,
            tc.tile_pool(name="work", bufs=1) as work,
            tc.tile_pool(name="ps", bufs=1, space="PSUM") as ps,
        ):
            # ---- constants ----
            ones64 = cpool.tile([128, 64], F32)      # row 64 = 1.0 (bcast lhsT)
            nc.gpsimd.memset(ones64[:], 0.0)
            nc.gpsimd.memset(ones64[64:65, :], 1.0)
            ones_r = cpool.tile([1, 128], F32)       # row 0 = 1.0 (bo bcast)
            nc.gpsimd.memset(ones_r[:], 1.0)
            tri = cpool.tile([128, 128], F32)        # 1 where col >= row
            nc.gpsimd.memset(tri[:], 1.0)
            nc.gpsimd.affine_select(
                out=tri[:], in_=tri[:], compare_op=mybir.AluOpType.is_ge,
                fill=0.0, base=0, pattern=[[1, 128]], channel_multiplier=-1)

            bo_row = cpool.tile([1, GC], F32)
            nc.sync.dma_start(bo_row[:], bo[:])
            bo_ps = ps.tile([128, GC], F32, tag="b", bufs=2)
            nc.tensor.matmul(bo_ps[:], ones_r[:, :], bo_row[:],
                             start=True, stop=True)
            bo_bc = cpool.tile([128, GC], F32)
            nc.scalar.copy(bo_bc[:], bo_ps[:])

            bq_sb = cpool.tile([128, 2, 1], F32)
            bk_sb = cpool.tile([128, 2, 1], F32)
            for ct in range(2):
                nc.sync.dma_start(bq_sb[:, ct, :], bq[ts(ct, 128), :])
                nc.sync.dma_start(bk_sb[:, ct, :], bk[ts(ct, 128), :])

            # ---- resident weights/activations ----
            xt_sb = big.tile([128, ND, S], F32R)
            wq_sb = big.tile([128, ND, GC], F32R)
            wk_sb = big.tile([128, ND, GC], F32R)
            wv_sb = big.tile([128, ND, GC], F32R)
            wo_sb = big.tile([128, ND, GC], F32R)
            for d in range(ND):
                nc.sync.dma_start(xt_sb[:, d, :], xt[ts(d, 128), :].bitcast(F32R))
                nc.sync.dma_start(wq_sb[:, d, :], wq[ts(d, 128), :].bitcast(F32R))
                nc.sync.dma_start(wk_sb[:, d, :], wk[ts(d, 128), :].bitcast(F32R))
                nc.sync.dma_start(wv_sb[:, d, :], wv[ts(d, 128), :].bitcast(F32R))
                nc.sync.dma_start(wo_sb[:, d, :], wo[ts(d, 128), :].bitcast(F32R))

            qt_sb = big.tile([128, 2, S], F32R)   # QT/8 (+bq/8): [col, s]
            kt_sb = big.tile([128, 2, S], F32R)
            v_sb = big.tile([128, NKT, HPG, HEAD_DIM + 1], F32R)  # V | ones
            ones_v = cpool.tile([128, NKT, HPG, 1], F32)
            nc.gpsimd.memset(ones_v[:], 1.0)
            nc.vector.tensor_copy(v_sb[:, :, :, HEAD_DIM:HEAD_DIM + 1], ones_v[:])

            def body():
                # ---- QKV projections ----
                for ct in range(2):
                    for sb in range(NQB):
                        pq = ps.tile([128, QB], F32, tag="s", bufs=3)
                        for d in range(ND):
                            nc.tensor.matmul(
                                pq[:], wq_sb[:, d, ts(ct, 128)],
                                xt_sb[:, d, ts(sb, QB)],
                                start=(d == 0), stop=(d == ND - 1))
                        nc.scalar.activation(
                            qt_sb[:, ct, ts(sb, QB)], pq[:], AF.Identity,
                            bias=bq_sb[:, ct, :], scale=0.125)
                        pk = ps.tile([128, QB], F32, tag="s", bufs=3)
                        for d in range(ND):
                            nc.tensor.matmul(
                                pk[:], wk_sb[:, d, ts(ct, 128)],
                                xt_sb[:, d, ts(sb, QB)],
                                start=(d == 0), stop=(d == ND - 1))
                        nc.scalar.activation(
                            kt_sb[:, ct, ts(sb, QB)], pk[:], AF.Identity,
                            bias=bk_sb[:, ct, :], scale=1.0)
                for st in range(NKT):
                    pv = ps.tile([128, GC], F32, tag="b", bufs=2)
                    for d in range(ND):
                        nc.tensor.matmul(
                            pv[:], xt_sb[:, d, ts(st, 128)],
                            wv_sb[:, d, :],
                            start=(d == 0), stop=(d == ND - 1))
                    nc.vector.tensor_copy(
                        v_sb[:, st, :, 0:HEAD_DIM],
                        pv[:].rearrange("p (h d) -> p h d", h=HPG))

                # ---- attention + collective + output projection ----
                for qb in range(NQB):
                    nkt_q = 4 * qb + 4       # causal: k tiles 0..4qb+3
                    for ct in range(2):
                        for hl in range(2):
                            h = ct * 2 + hl
                            pr = slice(64 * hl, 64 * hl + 64)
                            po = ps.tile([65, QB], F32, tag="o", bufs=2)
                            for kt in range(nkt_q):
                                dg = kt - 4 * qb
                                c0 = max(dg, 0) * 128
                                cols = slice(c0, QB)
                                pst = ps.tile([128, QB], F32, tag="s", bufs=3)
                                nc.tensor.matmul(
                                    pst[:, cols],
                                    kt_sb[pr, ct, ts(kt, KT)],
                                    qt_sb[pr, ct, qb * QB + c0:(qb + 1) * QB],
                                    start=True, stop=True,
                                    tile_position=(64 * hl, 0))
                                pt = work.tile([128, QB], F32R, tag="pt", bufs=4)
                                nc.scalar.activation(
                                    pt[:, cols], pst[:, cols], AF.Exp)
                                if dg >= 0:
                                    nc.vector.tensor_mul(
                                        pt[:, c0:c0 + 128],
                                        pt[:, c0:c0 + 128], tri[:])
                                if debug and qb == 0 and h == 0 and kt == 0:
                                    nc.sync.dma_start(dbg_pt[:], pt[:].bitcast(F32))
                                nc.tensor.matmul(
                                    po[:, cols], v_sb[:, kt, h, :],
                                    pt[:, cols],
                                    start=(kt == 0), stop=(kt == nkt_q - 1),
                                    skip_group_check=True)
                            at = work.tile([65, QB], F32, tag="at", bufs=3)
                            if debug and qb == 0 and h == 0:
                                po_sb = work.tile([65, QB], F32, tag="dbg", bufs=1)
                                nc.scalar.copy(po_sb[:], po[:])
                                nc.sync.dma_start(dbg_po[:], po_sb[:])
                            nc.scalar.copy(at[0:64, :], po[0:64, :])
                            rc = work.tile([65, QB], F32, tag="rc", bufs=2)
                            nc.vector.reciprocal_approx_fast(rc[:], po[:])
                            pb = ps.tile([64, QB], F32, tag="b", bufs=2)
                            nc.tensor.matmul(
                                pb[:], ones64[64:65, 0:64], rc[64:65, :],
                                start=True, stop=True)
                            nc.vector.tensor_mul(at[0:64, :], at[0:64, :], pb[:])
                            if debug and qb == 0 and h == 0:
                                nc.sync.dma_start(dbg_at[0:64, :], at[0:64, :])
                                nc.sync.dma_start(dbg_at[64:65, :], rc[64:65, :])
                            nc.sync.dma_start(
                                loc[qb][ts(h, 64), :], at[0:64, :])
                    nc.gpsimd.collective_compute(
                        "AllGather", mybir.AluOpType.bypass,
                        replica_groups=REPLICA_GROUPS,
                        ins=[loc[qb][:]], outs=[ag[qb][:]])
                    agt = work.tile([128, ND, QB], F32R, tag="ag", bufs=1)
                    for ft in range(ND):
                        nc.sync.dma_start(agt[:, ft, :], ag[qb][ts(ft, 128), :].bitcast(F32R))
                    for st4 in range(QB // 128):
                        pp = ps.tile([128, GC], F32, tag="o", bufs=2)
                        for ft in range(ND):
                            nc.tensor.matmul(
                                pp[:], agt[:, ft, ts(st4, 128)],
                                wo_sb[:, ft, :],
                                start=(ft == 0), stop=(ft == ND - 1))
                        ot = work.tile([128, GC], F32, tag="ot", bufs=3)
                        nc.vector.tensor_add(ot[:], pp[:], bo_bc[:])
                        nc.sync.dma_start(
                            out[qb * QB + st4 * 128:qb * QB + (st4 + 1) * 128, :],
                            ot[:])

            if repeat == 1:
                body()
                if debug:
                    nc.sync.dma_start(dbg_qt[:], qt_sb[:].rearrange("p a b -> p (a b)").bitcast(F32))
                    nc.sync.dma_start(dbg_kt[:], kt_sb[:].rearrange("p a b -> p (a b)").bitcast(F32))
                    nc.sync.dma_start(dbg_v[:], v_sb[:].rearrange("p a b c -> p (a b c)").bitcast(F32))
            else:
                with tc.For_i(0, repeat, 1):
                    body()

    nc.compile()
    return nc


def shard_inputs(X, Wq, bq, Wk, bk, Wv, bv, Wo, bo):
    """Full inputs -> per-core input maps."""
    in_maps = []
    for c in range(N_CORES):
        b, g = c // N_GROUPS, c % N_GROUPS
        cs = slice(g * GC, (g + 1) * GC)
        bo_eff = bo[cs] + bv.astype(np.float64) @ Wo[:, cs].astype(np.float64)
        in_maps.append({
            "xt": np.ascontiguousarray(X[b].T),
            "wq": np.ascontiguousarray(Wq[:, cs]),
            "wk": np.ascontiguousarray(Wk[:, cs]),
            "wv": np.ascontiguousarray(Wv[:, cs]),
            "wo": np.ascontiguousarray(Wo[:, cs]),
            "bq": np.ascontiguousarray((bq[cs] / 8.0).reshape(GC, 1)),
            "bk": np.ascontiguousarray(bk[cs].reshape(GC, 1)),
            "bo": np.ascontiguousarray(bo_eff.astype(np.float32).reshape(1, GC)),
        })
    return in_maps


def assemble_output(results):
    """Per-core output maps -> full [B, S, D]."""
    full = np.empty((B, S, MODEL_DIM), dtype=np.float32)
    for c in range(N_CORES):
        b, g = c // N_GROUPS, c % N_GROUPS
        full[b, :, g * GC:(g + 1) * GC] = results[c]["out"]
    return full


_prog_cache = {}


def kernel(X, Wq, bq, Wk, bk, Wv, bv, Wo, bo):
    X = np.asarray(X, dtype=np.float32)
    args = [np.asarray(a, dtype=np.float32)
            for a in (Wq, bq, Wk, bk, Wv, bv, Wo, bo)]
    if "nc" not in _prog_cache:
        _prog_cache["nc"] = build_program(repeat=1)
    nc = _prog_cache["nc"]
    in_maps = shard_inputs(X, *args)
    res = run_bass_kernel_spmd(nc, in_maps, list(range(N_CORES)))
    return assemble_output(res.results)


# revision 11
# speedup vs baseline: 1.6172x; 1.2730x over previous
"""Causal multi-head attention (B=2, S=2048, D=1024, H=16) on 8 Trainium2
NeuronCores.

Sharding: core c = (batch b = c//4) x (head-group g = c%4, 4 heads each).
Per core:
  - QKV projections for its 4 heads, computed in transposed layout
    (QT/KT: [head-cols, S]) plus V in natural layout [S, head-cols].
  - Flash-style causal attention per head with scores computed transposed
    (ST[k, q]) so softmax normalization lands on PE/ACT/DVE cheaply:
    exp without max-subtraction (scores are O(1) by construction), row sums
    via an appended ones-column on V, division via reciprocal_approx_fast
    plus a PE broadcast.
  - AllGather (groups of 4 cores sharing a batch) of the normalized,
    transposed attention output, then a column-parallel output projection.
Host side only shards inputs / concatenates outputs.

All matmuls run as float32r (1-pass fp32, ~FP22 multiply, fp32 accumulate).
"""
import sys

if "/opt/trn_rl_repo" not in sys.path:
    sys.path.insert(0, "/opt/trn_rl_repo")

import numpy as np

import concourse.bass as bass
import concourse.mybir as mybir
import concourse.tile as tile
from concourse import bacc
from concourse.bass import ts
from concourse.bass_utils import run_bass_kernel_spmd

F32 = mybir.dt.float32
F32R = mybir.dt.float32r
AF = mybir.ActivationFunctionType

MODEL_DIM = 1024
NUM_HEADS = 16
HEAD_DIM = 64
B = 2
S = 2048
N_CORES = 8
N_GROUPS = 4            # head groups (tensor parallel within a batch)
HPG = NUM_HEADS // N_GROUPS   # heads per group = 4
GC = HPG * HEAD_DIM     # head-group feature width = 256
QB = 512                # query block (PSUM bank free size)
NQB = S // QB           # 4
KT = 128                # key tile
NKT = S // KT           # 16
ND = MODEL_DIM // 128   # 8 contraction tiles for the projections

REPLICA_GROUPS = [[0, 1, 2, 3], [4, 5, 6, 7]]




def build_program(repeat: int = 1, debug: bool = False):
    """Build the SPMD per-core program. `repeat` re-runs the whole body
    (identical work) for wall-clock loop-differenced timing."""
    nc = bacc.Bacc("TRN2", num_devices=N_CORES)

    xt = nc.dram_tensor("xt", [MODEL_DIM, S], F32, kind="ExternalInput")
    wq = nc.dram_tensor("wq", [MODEL_DIM, GC], F32, kind="ExternalInput")
    wk = nc.dram_tensor("wk", [MODEL_DIM, GC], F32, kind="ExternalInput")
    wv = nc.dram_tensor("wv", [MODEL_DIM, GC], F32, kind="ExternalInput")
    wo = nc.dram_tensor("wo", [MODEL_DIM, GC], F32, kind="ExternalInput")
    bq = nc.dram_tensor("bq", [GC, 1], F32, kind="ExternalInput")   # already /8
    bk = nc.dram_tensor("bk", [GC, 1], F32, kind="ExternalInput")
    bo = nc.dram_tensor("bo", [1, GC], F32, kind="ExternalInput")   # bo + bv@Wo
    out = nc.dram_tensor("out", [S, GC], F32, kind="ExternalOutput")

    # internal DRAM for the collective, one pair per query block
    loc = [nc.dram_tensor(f"loc{i}", [GC, QB], F32) for i in range(NQB)]
    ag = [nc.dram_tensor(f"ag{i}", [MODEL_DIM, QB], F32) for i in range(NQB)]

    if debug:
        dbg_qt = nc.dram_tensor("dbg_qt", [128, 2 * S], F32, kind="ExternalOutput")
        dbg_kt = nc.dram_tensor("dbg_kt", [128, 2 * S], F32, kind="ExternalOutput")
        dbg_v = nc.dram_tensor("dbg_v", [128, NKT * HPG * (HEAD_DIM + 1)], F32,
                               kind="ExternalOutput")
        dbg_po = nc.dram_tensor("dbg_po", [65, QB], F32, kind="ExternalOutput")
        dbg_at = nc.dram_tensor("dbg_at", [65, QB], F32, kind="ExternalOutput")
        dbg_pt = nc.dram_tensor("dbg_pt", [128, QB], F32, kind="ExternalOutput")

    with tile.TileContext(nc) as tc:
        with (
            tc.tile_pool(name="const", bufs=1) as cpool,
            tc.tile_pool(name="big", bufs=1) as big,
            tc.tile_pool(name="work", bufs=1) as work,
            tc.tile_pool(name="ps", bufs=1, space="PSUM") as ps,
        ):
            # ---- constants ----
            ones64 = cpool.tile([128, 64], F32)      # row 64 = 1.0 (bcast lhsT)
            nc.gpsimd.memset(ones64[:], 0.0)
            nc.gpsimd.memset(ones64[64:65, :], 1.0)
            ones_r = cpool.tile([1, 128], F32)       # row 0 = 1.0 (bo bcast)
            nc.gpsimd.memset(ones_r[:], 1.0)
            tri = cpool.tile([128, 128], F32)        # 1 where col >= row
            nc.gpsimd.memset(tri[:], 1.0)
            nc.gpsimd.affine_select(
                out=tri[:], in_=tri[:], compare_op=mybir.AluOpType.is_ge,
                fill=0.0, base=0, pattern=[[1, 128]], channel_multiplier=-1)

            bo_row = cpool.tile([1, GC], F32)
            nc.sync.dma_start(bo_row[:], bo[:])
            bo_ps = ps.tile([128, GC], F32, tag="b", bufs=1)
            nc.tensor.matmul(bo_ps[:], ones_r[:, :], bo_row[:],
                             start=True, stop=True)
            bo_bc = cpool.tile([128, GC], F32)
            nc.scalar.copy(bo_bc[:], bo_ps[:])

            bq_sb = cpool.tile([128, 2, 1], F32)
            bk_sb = cpool.tile([128, 2, 1], F32)
            for ct in range(2):
                nc.sync.dma_start(bq_sb[:, ct, :], bq[ts(ct, 128), :])
                nc.sync.dma_start(bk_sb[:, ct, :], bk[ts(ct, 128), :])

            # ---- resident weights/activations ----
            xt_sb = big.tile([128, ND, S], F32R)
            wq_sb = big.tile([128, ND, GC], F32R)
            wk_sb = big.tile([128, ND, GC], F32R)
            wv_sb = big.tile([128, ND, GC], F32R)
            wo_sb = big.tile([128, ND, GC], F32R)
            for d in range(ND):
                nc.sync.dma_start(xt_sb[:, d, :], xt[ts(d, 128), :].bitcast(F32R))
                nc.sync.dma_start(wq_sb[:, d, :], wq[ts(d, 128), :].bitcast(F32R))
                nc.sync.dma_start(wk_sb[:, d, :], wk[ts(d, 128), :].bitcast(F32R))
                nc.sync.dma_start(wv_sb[:, d, :], wv[ts(d, 128), :].bitcast(F32R))
                nc.sync.dma_start(wo_sb[:, d, :], wo[ts(d, 128), :].bitcast(F32R))

            qt_sb = big.tile([128, 2, S], F32R)   # QT/8 (+bq/8): [col, s]
            kt_sb = big.tile([128, 2, S], F32R)
            v_sb = big.tile([128, NKT, HPG, HEAD_DIM + 1], F32R)  # V | ones
            ones_v = cpool.tile([128, NKT, HPG, 1], F32)
            nc.gpsimd.memset(ones_v[:], 1.0)
            nc.vector.tensor_copy(v_sb[:, :, :, HEAD_DIM:HEAD_DIM + 1], ones_v[:])

            def body():
                # ---- QKV projections ----
                for ct in range(2):
                    for sb in range(NQB):
                        pq = ps.tile([128, QB], F32, tag="s", bufs=3)
                        for d in range(ND):
                            nc.tensor.matmul(
                                pq[:], wq_sb[:, d, ts(ct, 128)],
                                xt_sb[:, d, ts(sb, QB)],
                                start=(d == 0), stop=(d == ND - 1))
                        nc.scalar.activation(
                            qt_sb[:, ct, ts(sb, QB)], pq[:], AF.Identity,
                            bias=bq_sb[:, ct, :], scale=0.125)
                        pk = ps.tile([128, QB], F32, tag="s", bufs=3)
                        for d in range(ND):
                            nc.tensor.matmul(
                                pk[:], wk_sb[:, d, ts(ct, 128)],
                                xt_sb[:, d, ts(sb, QB)],
                                start=(d == 0), stop=(d == ND - 1))
                        nc.scalar.activation(
                            kt_sb[:, ct, ts(sb, QB)], pk[:], AF.Identity,
                            bias=bk_sb[:, ct, :], scale=1.0)
                for st in range(NKT):
                    pv = ps.tile([128, GC], F32, tag="b", bufs=1)
                    for d in range(ND):
                        nc.tensor.matmul(
                            pv[:], xt_sb[:, d, ts(st, 128)],
                            wv_sb[:, d, :],
                            start=(d == 0), stop=(d == ND - 1))
                    nc.vector.tensor_copy(
                        v_sb[:, st, :, 0:HEAD_DIM],
                        pv[:].rearrange("p (h d) -> p h d", h=HPG))

                # ---- attention + collective + output projection ----
                def outproj(qb):
                    agt = work.tile([128, ND, QB], F32R, tag="ag", bufs=1)
                    for ft in range(ND):
                        nc.sync.dma_start(agt[:, ft, :], ag[qb][ts(ft, 128), :].bitcast(F32R))
                    for st4 in range(QB // 128):
                        pp = ps.tile([128, GC], F32, tag="p", bufs=2)
                        for ft in range(ND):
                            nc.tensor.matmul(
                                pp[:], agt[:, ft, ts(st4, 128)],
                                wo_sb[:, ft, :],
                                start=(ft == 0), stop=(ft == ND - 1))
                        ot = work.tile([128, GC], F32, tag="ot", bufs=3)
                        nc.vector.tensor_add(ot[:], pp[:], bo_bc[:])
                        nc.sync.dma_start(
                            out[qb * QB + st4 * 128:qb * QB + (st4 + 1) * 128, :],
                            ot[:])

                for qb in range(NQB):
                    nkt_q = 4 * qb + 4       # causal: k tiles 0..4qb+3
                    for ct in range(2):
                        for hl in range(2):
                            h = ct * 2 + hl
                            pr = slice(64 * hl, 64 * hl + 64)
                            po = ps.tile([65, QB], F32, tag="o", bufs=2)
                            for kt in range(nkt_q):
                                dg = kt - 4 * qb
                                c0 = max(dg, 0) * 128
                                cols = slice(c0, QB)
                                pst = ps.tile([128, QB], F32, tag="s", bufs=3)
                                nc.tensor.matmul(
                                    pst[:, cols],
                                    kt_sb[pr, ct, ts(kt, KT)],
                                    qt_sb[pr, ct, qb * QB + c0:(qb + 1) * QB],
                                    start=True, stop=True,
                                    tile_position=(64 * hl, 0))
                                pt = work.tile([128, QB], F32R, tag="pt", bufs=4)
                                nc.scalar.activation(
                                    pt[:, cols], pst[:, cols], AF.Exp)
                                if dg >= 0:
                                    nc.vector.tensor_mul(
                                        pt[:, c0:c0 + 128],
                                        pt[:, c0:c0 + 128], tri[:])
                                if debug and qb == 0 and h == 0 and kt == 0:
                                    nc.sync.dma_start(dbg_pt[:], pt[:].bitcast(F32))
                                nc.tensor.matmul(
                                    po[:, cols], v_sb[:, kt, h, :],
                                    pt[:, cols],
                                    start=(kt == 0), stop=(kt == nkt_q - 1),
                                    skip_group_check=True)
                            at = work.tile([65, QB], F32, tag="at", bufs=3)
                            if debug and qb == 0 and h == 0:
                                po_sb = work.tile([65, QB], F32, tag="dbg", bufs=1)
                                nc.scalar.copy(po_sb[:], po[:])
                                nc.sync.dma_start(dbg_po[:], po_sb[:])
                            nc.scalar.copy(at[0:64, :], po[0:64, :])
                            rc = work.tile([65, QB], F32, tag="rc", bufs=2)
                            nc.vector.reciprocal_approx_fast(rc[:], po[:])
                            pb = ps.tile([64, QB], F32, tag="b", bufs=1)
                            nc.tensor.matmul(
                                pb[:], ones64[64:65, 0:64], rc[64:65, :],
                                start=True, stop=True)
                            nc.vector.tensor_mul(at[0:64, :], at[0:64, :], pb[:])
                            if debug and qb == 0 and h == 0:
                                nc.sync.dma_start(dbg_at[0:64, :], at[0:64, :])
                                nc.sync.dma_start(dbg_at[64:65, :], rc[64:65, :])
                            nc.sync.dma_start(
                                loc[qb][ts(h, 64), :], at[0:64, :])
                    nc.gpsimd.collective_compute(
                        "AllGather", mybir.AluOpType.bypass,
                        replica_groups=REPLICA_GROUPS,
                        ins=[loc[qb][:]], outs=[ag[qb][:]])
                    if qb >= 1:
                        outproj(qb - 1)
                outproj(NQB - 1)

            if repeat == 1:
                body()
                if debug:
                    nc.sync.dma_start(dbg_qt[:], qt_sb[:].rearrange("p a b -> p (a b)").bitcast(F32))
                    nc.sync.dma_start(dbg_kt[:], kt_sb[:].rearrange("p a b -> p (a b)").bitcast(F32))
                    nc.sync.dma_start(dbg_v[:], v_sb[:].rearrange("p a b c -> p (a b c)").bitcast(F32))
            else:
                with tc.For_i(0, repeat, 1):
                    body()

    nc.compile()
    return nc


def shard_inputs(X, Wq, bq, Wk, bk, Wv, bv, Wo, bo):
    """Full inputs -> per-core input maps."""
    in_maps = []
    for c in range(N_CORES):
        b, g = c // N_GROUPS, c % N_GROUPS
        cs = slice(g * GC, (g + 1) * GC)
        bo_eff = bo[cs] + bv.astype(np.float64) @ Wo[:, cs].astype(np.float64)
        in_maps.append({
            "xt": np.ascontiguousarray(X[b].T),
            "wq": np.ascontiguousarray(Wq[:, cs]),
            "wk": np.ascontiguousarray(Wk[:, cs]),
            "wv": np.ascontiguousarray(Wv[:, cs]),
            "wo": np.ascontiguousarray(Wo[:, cs]),
            "bq": np.ascontiguousarray((bq[cs] / 8.0).reshape(GC, 1)),
            "bk": np.ascontiguousarray(bk[cs].reshape(GC, 1)),
            "bo": np.ascontiguousarray(bo_eff.astype(np.float32).reshape(1, GC)),
        })
    return in_maps


def assemble_output(results):
    """Per-core output maps -> full [B, S, D]."""
    full = np.empty((B, S, MODEL_DIM), dtype=np.float32)
    for c in range(N_CORES):
        b, g = c // N_GROUPS, c % N_GROUPS
        full[b, :, g * GC:(g + 1) * GC] = results[c]["out"]
    return full


_prog_cache = {}


def kernel(X, Wq, bq, Wk, bk, Wv, bv, Wo, bo):
    X = np.asarray(X, dtype=np.float32)
    args = [np.asarray(a, dtype=np.float32)
            for a in (Wq, bq, Wk, bk, Wv, bv, Wo, bo)]
    if "nc" not in _prog_cache:
        _prog_cache["nc"] = build_program(repeat=1)
    nc = _prog_cache["nc"]
    in_maps = shard_inputs(X, *args)
    res = run_bass_kernel_spmd(nc, in_maps, list(range(N_CORES)))
    return assemble_output(res.results)
